# revision 12
# baseline (speedup 1.0000x reference)
"""ChamferNormalLoss Trainium2 kernel.

Strategy (data-parallel over batch, 2 batches per core x 8 cores):
  device per batch:
    - PE matmul computes proxy[p,g] = 2*p.g - |g|^2  (argmax proxy == argmin
      of chamfer d2) into PSUM, 16 p-tiles x 16 g-banks of [128,512].
    - ACT copies each PSUM bank to an fp16 shadow row [128, 8192] in SBUF.
    - DVE InstMax + InstMaxIndex extract the argmax index per pred vertex.
    - near indices roundtrip through DRAM to build the wrapped idx layout,
      gpsimd ap_gather picks the nearest gt vertex normals from an SBUF table.
    - gathered normals are renormalized, dotted against host-pregrouped
      (by edge-endpoint e0) pred edge vectors, |.| and reduced.
  host: topology preprocessing (edge grouping, gt augmentation), gt vertex
  normal table construction, final mean across cores/partitions.
"""

import os

import numpy as np

B, NP, NG = 16, 2048, 8192
NCORES = 8
BPC = B // NCORES  # batches per core
PT = NP // 128  # p tiles per batch (16)
GB = NG // 512  # g banks per batch (16)

_prog_cache = {}


def _host_normals(gt, faces):
    """Per-batch gt vertex normals, matching reference _vertex_normals+_normalize."""
    out = np.zeros((gt.shape[0], NG, 4), dtype=np.float32)
    f0, f1, f2 = faces[:, 0], faces[:, 1], faces[:, 2]
    for b in range(gt.shape[0]):
        gv = gt[b]
        fn = np.cross(gv[f1] - gv[f0], gv[f2] - gv[f0]).astype(np.float32)
        vn = np.zeros((NG, 3), dtype=np.float32)
        np.add.at(vn, f0, fn)
        np.add.at(vn, f1, fn)
        np.add.at(vn, f2, fn)
        n = np.sqrt((vn * vn).sum(-1, keepdims=True))
        vn = vn / np.maximum(n, 1e-12)
        out[b, :, :3] = vn
    return out


def _host_prep(pred, gt, edges, faces):
    e0 = edges[:, 0].astype(np.int64)
    e1 = edges[:, 1].astype(np.int64)
    E = e0.shape[0]

    # group edges by e0 into padded slots
    order = np.argsort(e0, kind="stable")
    e0s, e1s = e0[order], e1[order]
    counts = np.bincount(e0s, minlength=NP)
    D = int(counts.max())
    D = max(D, 1)
    # slot index within each group
    starts = np.zeros(NP, dtype=np.int64)
    starts[1:] = np.cumsum(counts)[:-1]
    slot = np.arange(E) - starts[e0s]

    # pedge[b, p, t, j, d]: edge vectors grouped by e0 vertex
    pedge = np.zeros((B, NP, D, 4), dtype=np.float32)
    ev = pred[:, e0s, :] - pred[:, e1s, :]  # [B, E, 3]
    pedge[:, e0s, slot, :3] = ev
    pedge = pedge.reshape(B, PT, 128, D, 4).transpose(0, 2, 1, 3, 4).copy()

    # paug rows [2px, 2py, 2pz, -1, -|p|^2]; gaug rows [gx, gy, gz, |g|^2, 1]
    # so PE computes -d2 = 2 p.g - |g|^2 - |p|^2 (tiny near the argmax -> fp16 safe)
    paug = np.empty((B, 5, NP), dtype=np.float32)
    paug[:, :3] = 2.0 * pred.transpose(0, 2, 1)
    paug[:, 3] = -1.0
    paug[:, 4] = -(pred * pred).sum(-1)
    gaug = np.empty((B, 5, NG), dtype=np.float32)
    gaug[:, :3] = gt.transpose(0, 2, 1)
    gaug[:, 3] = (gt * gt).sum(-1)
    gaug[:, 4] = 1.0

    vn16 = _host_normals(gt, faces).astype(np.float16)  # [B, NG, 4]
    return paug, gaug, vn16, pedge, D, E


def _build_program(D):
    import concourse.bacc as bacc
    import concourse.bass as bass  # noqa: F401
    import concourse.mybir as mybir
    import concourse.tile as tile

    f32 = mybir.dt.float32
    f16 = mybir.dt.float16
    u16 = mybir.dt.uint16

    nc = bacc.Bacc("TRN2", target_bir_lowering=False)

    paug_d = nc.dram_tensor("paug", [BPC, 5, NP], f32, kind="ExternalInput")
    gaug_d = nc.dram_tensor("gaug", [BPC, 5, NG], f32, kind="ExternalInput")
    vn_d = nc.dram_tensor("vn16", [BPC, NG, 4], f16, kind="ExternalInput")
    pedge_d = nc.dram_tensor("pedge", [BPC, 128, PT, D, 4], f32, kind="ExternalInput")
    loss_d = nc.dram_tensor("loss", [BPC, 128, 1], f32, kind="ExternalOutput")
    nidx_d = nc.dram_tensor("nidx_scratch", [BPC, NP], u16, kind="Internal")
    nn_d = nc.dram_tensor("nn_scratch", [BPC, NP, 4], f16, kind="Internal")

    with tile.TileContext(nc) as tc:
        with (
            tc.tile_pool(name="consts", bufs=1) as consts,
            tc.tile_pool(name="shadow", bufs=2) as shadow_pool,
            tc.tile_pool(name="psum", bufs=8, space="PSUM") as psum_pool,
            tc.tile_pool(name="small", bufs=4) as small,
            tc.tile_pool(name="work", bufs=2) as work,
        ):
            for b in range(BPC):
                gaug_t = consts.tile([5, NG], f32, tag="gaug")
                nc.sync.dma_start(out=gaug_t, in_=gaug_d[b])
                paug_t = consts.tile([5, NP], f32, tag="paug")
                nc.sync.dma_start(out=paug_t, in_=paug_d[b])
                # vn table broadcast to partitions 0..15
                vn_t = consts.tile([16, NG, 4], f16, tag="vn")
                nc.sync.dma_start(
                    out=vn_t, in_=vn_d[b].unsqueeze(0).partition_broadcast(16)
                )

                nearidx = work.tile([128, PT, 8], u16, tag="nearidx")
                for t in range(PT):
                    sh = shadow_pool.tile([128, NG], f16, tag="shadow")
                    for c in range(GB):
                        ps = psum_pool.tile([128, 512], f32, tag="ps")
                        nc.tensor.matmul(
                            ps,
                            lhsT=paug_t[:, t * 128 : (t + 1) * 128],
                            rhs=gaug_t[:, c * 512 : (c + 1) * 512],
                            start=True,
                            stop=True,
                        )
                        nc.scalar.activation(
                            out=sh[:, c * 512 : (c + 1) * 512],
                            in_=ps,
                            func=mybir.ActivationFunctionType.Copy,
                        )
                    mx = small.tile([128, 8], f16, tag="mx")
                    nc.vector.max(mx, sh)
                    nc.vector.max_index(nearidx[:, t, :], mx, sh)

                # scatter indices to DRAM in natural g order: g = t*128 + p
                nc.sync.dma_start(
                    out=nidx_d[b].rearrange("(t p) -> p t", p=128),
                    in_=nearidx[:, :, 0],
                )
                # read back wrapped for ap_gather (channels=16):
                # idxw[p, s] = nearidx_dram[s*16 + p]
                idxw = small.tile([16, NP // 16], u16, tag="idxw")
                nc.sync.dma_start(
                    out=idxw, in_=nidx_d[b].rearrange("(s p) -> p s", p=16)
                )
                nn_rep = work.tile([16, NP, 4], f16, tag="nnrep")
                nc.gpsimd.ap_gather(
                    out_ap=nn_rep,
                    in_ap=vn_t,
                    idxs_ap=idxw[:, :].bitcast(mybir.dt.int16),
                    channels=16,
                    num_elems=NG,
                    d=4,
                    num_idxs=NP,
                )
                # roundtrip to get [128, PT, 4] layout (partition p holds g=t*128+p)
                nc.sync.dma_start(out=nn_d[b].unsqueeze(0), in_=nn_rep[0:1])
                nn_t = work.tile([128, PT, 4], f16, tag="nnt")
                nc.sync.dma_start(
                    out=nn_t, in_=nn_d[b].rearrange("(t p) d -> p t d", p=128)
                )

                # renormalize gathered normals
                sq = small.tile([128, PT, 4], f32, tag="sq")
                nc.vector.tensor_mul(sq, nn_t, nn_t)
                ss = small.tile([128, PT], f32, tag="ss")
                nc.vector.tensor_reduce(
                    out=ss, in_=sq, axis=mybir.AxisListType.X, op=mybir.AluOpType.add
                )
                nrm = small.tile([128, PT], f32, tag="nrm")
                nc.scalar.activation(
                    out=nrm, in_=ss, func=mybir.ActivationFunctionType.Sqrt
                )
                nc.vector.tensor_scalar_max(nrm, nrm, 1e-12)
                rs = small.tile([128, PT], f32, tag="rs")
                nc.vector.reciprocal(rs, nrm)
                nnn = work.tile([128, PT, 4], f32, tag="nnn")
                nc.vector.tensor_mul(
                    nnn, nn_t, rs[:, :].unsqueeze(-1).broadcast_to([128, PT, 4])
                )

                # dots with pre-grouped edge vectors, |.|, reduce
                pedge_t = work.tile([128, PT, D, 4], f32, tag="pedge")
                nc.sync.dma_start(out=pedge_t, in_=pedge_d[b])
                prod = work.tile([128, PT, D, 4], f32, tag="prod")
                nc.vector.tensor_mul(
                    prod,
                    pedge_t,
                    nnn[:, :, :].unsqueeze(2).broadcast_to([128, PT, D, 4]),
                )
                dots = small.tile([128, PT, D], f32, tag="dots")
                nc.vector.tensor_reduce(
                    out=dots, in_=prod, axis=mybir.AxisListType.X, op=mybir.AluOpType.add
                )
                absd = small.tile([128, PT, D], f32, tag="absd")
                nc.scalar.activation(
                    out=absd, in_=dots, func=mybir.ActivationFunctionType.Abs
                )
                lp = small.tile([128, 1], f32, tag="lp")
                nc.vector.tensor_reduce(
                    out=lp, in_=absd, axis=mybir.AxisListType.XY, op=mybir.AluOpType.add
                )
                nc.sync.dma_start(out=loss_d[b], in_=lp)

    nc.compile()
    return nc


def _ensure_ntff_hook():
    """Register the axon NTFF profile hook if the image's antenv lacks it."""
    import contextlib
    import ctypes
    import sys
    import types

    try:
        from antenv.axon_hooks import get_axon_ntff_profile_hook  # noqa: F401

        return
    except ImportError:
        pass

    so_path = "/opt/axon/libaxon_pjrt.so"
    if not os.path.exists(so_path):
        return
    lib = ctypes.CDLL(so_path)
    if not hasattr(lib, "axon_start_nrt_profile"):
        return
    lib.axon_start_nrt_profile.argtypes = [
        ctypes.POINTER(ctypes.c_int64),
        ctypes.c_size_t,
    ]
    lib.axon_start_nrt_profile.restype = ctypes.c_int64
    lib.axon_stop_nrt_profile.argtypes = [ctypes.c_char_p]
    lib.axon_stop_nrt_profile.restype = ctypes.c_int64

    @contextlib.contextmanager
    def _hook(output_dir, device_ids):
        import jax

        jax.devices()
        if device_ids:
            ids = (ctypes.c_int64 * len(device_ids))(*device_ids)
            rc = lib.axon_start_nrt_profile(ids, len(device_ids))
        else:
            rc = lib.axon_start_nrt_profile(None, 0)
        if rc != 0:
            raise RuntimeError(f"axon_start_nrt_profile rc={rc}")
        try:
            yield
        finally:
            n = lib.axon_stop_nrt_profile(str(output_dir).encode())
            print(f"ntff profile: {n} file(s) written to {output_dir}", file=sys.stderr)

    mod = types.ModuleType("antenv.axon_hooks")
    holder = {"hook": _hook}
    mod.set_axon_ntff_profile_hook = lambda h: holder.__setitem__("hook", h)
    mod.get_axon_ntff_profile_hook = lambda: holder["hook"]
    sys.modules["antenv.axon_hooks"] = mod
    import antenv

    antenv.axon_hooks = mod


def kernel(pred_vertices, gt_vertices, edges, gt_faces):
    pred = np.asarray(pred_vertices, dtype=np.float32)
    gt = np.asarray(gt_vertices, dtype=np.float32)
    edges = np.asarray(edges)
    faces = np.asarray(gt_faces)

    paug, gaug, vn16, pedge, D, E = _host_prep(pred, gt, edges, faces)

    if D not in _prog_cache:
        _prog_cache[D] = _build_program(D)
    nc = _prog_cache[D]

    in_maps = []
    for c in range(NCORES):
        sl = slice(c * BPC, (c + 1) * BPC)
        in_maps.append(
            {
                "paug": np.ascontiguousarray(paug[sl]),
                "gaug": np.ascontiguousarray(gaug[sl]),
                "vn16": np.ascontiguousarray(vn16[sl]),
                "pedge": np.ascontiguousarray(pedge[sl]),
            }
        )

    from concourse.bass_utils import run_bass_kernel_spmd

    trace = bool(int(os.environ.get("CHAMFER_TRACE", "0")))
    if trace:
        _ensure_ntff_hook()
    res = run_bass_kernel_spmd(
        nc, in_maps, core_ids=list(range(NCORES)), trace=trace
    )
    if trace:
        kernel.last_results = res

    total = np.float64(0.0)
    for r in res.results:
        total += np.float64(r["loss"].sum(dtype=np.float64))
    loss = np.float32(total / (B * E))
    return np.asarray(loss, dtype=np.float32)


# revision 16
# speedup vs baseline: 1.6310x; 1.6310x over previous
"""ChamferNormalLoss Trainium2 kernel.

Strategy (data-parallel over batch, 2 batches per core x 8 cores):
  device per batch:
    - PE matmul computes proxy[p,g] = 2*p.g - |g|^2  (argmax proxy == argmin
      of chamfer d2) into PSUM, 16 p-tiles x 16 g-banks of [128,512].
    - ACT copies each PSUM bank to an fp16 shadow row [128, 8192] in SBUF.
    - DVE InstMax + InstMaxIndex extract the argmax index per pred vertex.
    - near indices roundtrip through DRAM to build the wrapped idx layout,
      gpsimd ap_gather picks the nearest gt vertex normals from an SBUF table.
    - gathered normals are renormalized, dotted against host-pregrouped
      (by edge-endpoint e0) pred edge vectors, |.| and reduced.
  host: topology preprocessing (edge grouping, gt augmentation), gt vertex
  normal table construction, final mean across cores/partitions.
"""

import os

import numpy as np

B, NP, NG = 16, 2048, 8192
NCORES = 8
BPC = B // NCORES  # batches per core
PT = NP // 128  # p tiles per batch (16)
GB = NG // 512  # g banks per batch (16)

_prog_cache = {}


def _host_normals(gt, faces):
    """Per-batch gt vertex normals, matching reference _vertex_normals+_normalize."""
    out = np.zeros((gt.shape[0], NG, 4), dtype=np.float32)
    f0, f1, f2 = faces[:, 0], faces[:, 1], faces[:, 2]
    for b in range(gt.shape[0]):
        gv = gt[b]
        fn = np.cross(gv[f1] - gv[f0], gv[f2] - gv[f0]).astype(np.float32)
        vn = np.zeros((NG, 3), dtype=np.float32)
        np.add.at(vn, f0, fn)
        np.add.at(vn, f1, fn)
        np.add.at(vn, f2, fn)
        n = np.sqrt((vn * vn).sum(-1, keepdims=True))
        vn = vn / np.maximum(n, 1e-12)
        out[b, :, :3] = vn
    return out


def _host_prep(pred, gt, edges, faces):
    e0 = edges[:, 0].astype(np.int64)
    e1 = edges[:, 1].astype(np.int64)
    E = e0.shape[0]

    # group edges by e0 into padded slots
    order = np.argsort(e0, kind="stable")
    e0s, e1s = e0[order], e1[order]
    counts = np.bincount(e0s, minlength=NP)
    D = int(counts.max())
    D = max(D, 1)
    # slot index within each group
    starts = np.zeros(NP, dtype=np.int64)
    starts[1:] = np.cumsum(counts)[:-1]
    slot = np.arange(E) - starts[e0s]

    # pedge[b, p, t, j, d]: edge vectors grouped by e0 vertex
    pedge = np.zeros((B, NP, D, 4), dtype=np.float32)
    ev = pred[:, e0s, :] - pred[:, e1s, :]  # [B, E, 3]
    pedge[:, e0s, slot, :3] = ev
    pedge = pedge.reshape(B, PT, 128, D, 4).transpose(0, 2, 1, 3, 4).copy()

    # bf16x3-split augmented operands so the PE computes
    #   psum = 2 p.g - |g|^2 - |p|^2 = -d2
    # in fast bf16 matmuls with ~f32 accuracy. Each f32 factor is split into
    # three bf16 planes (24 mantissa bits); row r pairs paug[r] * gaug[r].
    import ml_dtypes

    bf16 = ml_dtypes.bfloat16

    def split3(x):
        b0 = x.astype(bf16)
        r1 = x - b0.astype(np.float32)
        b1 = r1.astype(bf16)
        b2 = (r1 - b1.astype(np.float32)).astype(bf16)
        return b0, b1, b2

    KA = 24
    paug = np.zeros((B, KA, NP), dtype=bf16)
    gaug = np.zeros((B, KA, NG), dtype=bf16)
    for k in range(3):
        a0, a1, a2 = split3(2.0 * pred[:, :, k])
        g0, g1, g2 = split3(gt[:, :, k])
        pairs = [(a0, g0), (a0, g1), (a1, g0), (a0, g2), (a2, g0), (a1, g1)]
        for j, (pa, ga) in enumerate(pairs):
            paug[:, 6 * k + j] = pa
            gaug[:, 6 * k + j] = ga
    n0, n1, n2 = split3((gt * gt).sum(-1))
    for j, na in enumerate((n0, n1, n2)):
        paug[:, 18 + j] = np.float32(-1.0).astype(bf16)
        gaug[:, 18 + j] = na
    m0, m1, m2 = split3(-(pred * pred).sum(-1))
    for j, ma in enumerate((m0, m1, m2)):
        paug[:, 21 + j] = ma
        gaug[:, 21 + j] = np.float32(1.0).astype(bf16)

    vn16 = _host_normals(gt, faces).astype(np.float16)  # [B, NG, 4]
    return paug, gaug, vn16, pedge, D, E


def _build_program(D):
    import concourse.bacc as bacc
    import concourse.bass as bass  # noqa: F401
    import concourse.mybir as mybir
    import concourse.tile as tile

    f32 = mybir.dt.float32
    f16 = mybir.dt.float16
    bf16 = mybir.dt.bfloat16
    u16 = mybir.dt.uint16
    KA = 24

    nc = bacc.Bacc("TRN2", target_bir_lowering=False)

    paug_d = nc.dram_tensor("paug", [BPC, KA, NP], bf16, kind="ExternalInput")
    gaug_d = nc.dram_tensor("gaug", [BPC, KA, NG], bf16, kind="ExternalInput")
    vn_d = nc.dram_tensor("vn16", [BPC, NG, 4], f16, kind="ExternalInput")
    pedge_d = nc.dram_tensor("pedge", [BPC, 128, PT, D, 4], f32, kind="ExternalInput")
    loss_d = nc.dram_tensor("loss", [BPC, 128, 1], f32, kind="ExternalOutput")
    nidx_d = nc.dram_tensor("nidx_scratch", [BPC, NP], u16, kind="Internal")
    nn_d = nc.dram_tensor("nn_scratch", [BPC, NP, 4], f16, kind="Internal")

    with tile.TileContext(nc) as tc:
        with (
            tc.tile_pool(name="consts", bufs=1) as consts,
            tc.tile_pool(name="shadow", bufs=2) as shadow_pool,
            tc.tile_pool(name="psum", bufs=2, space="PSUM") as psum_pool,
            tc.tile_pool(name="small", bufs=4) as small,
            tc.tile_pool(name="work", bufs=2) as work,
        ):
            for b in range(BPC):
                gaug_t = consts.tile([KA, NG], bf16, tag="gaug")
                nc.sync.dma_start(out=gaug_t, in_=gaug_d[b])
                paug_t = consts.tile([KA, NP], bf16, tag="paug")
                nc.sync.dma_start(out=paug_t, in_=paug_d[b])
                # vn table broadcast to partitions 0..15
                vn_t = consts.tile([16, NG, 4], f16, tag="vn")
                nc.sync.dma_start(
                    out=vn_t, in_=vn_d[b].unsqueeze(0).partition_broadcast(16)
                )

                nearidx = work.tile([128, PT, 8], u16, tag="nearidx")
                for t in range(PT):
                    sh = shadow_pool.tile([128, NG], f16, tag="shadow")
                    lhsT = paug_t[:, t * 128 : (t + 1) * 128]
                    for c4 in range(GB // 4):
                        ps = psum_pool.tile([128, 2048], f32, tag="ps")
                        for q in range(4):
                            c = c4 * 4 + q
                            nc.tensor.matmul(
                                ps[:, q * 512 : (q + 1) * 512],
                                lhsT=lhsT,
                                rhs=gaug_t[:, c * 512 : (c + 1) * 512],
                                start=True,
                                stop=True,
                            )
                        # psum holds -d2; write +d2 to the fp16 shadow
                        nc.scalar.activation(
                            out=sh[:, c4 * 2048 : (c4 + 1) * 2048],
                            in_=ps,
                            func=mybir.ActivationFunctionType.Copy,
                            scale=-1.0,
                        )
                    # fp16 tensor-tensor min tree (2 elem/cycle) for the row min
                    tr = shadow_pool.tile([128, NG // 2], f16, tag="tree")
                    nc.vector.tensor_tensor(
                        out=tr,
                        in0=sh[:, : NG // 2],
                        in1=sh[:, NG // 2 :],
                        op=mybir.AluOpType.min,
                    )
                    n = NG // 4
                    while n >= 128:
                        nc.vector.tensor_tensor(
                            out=tr[:, :n],
                            in0=tr[:, :n],
                            in1=tr[:, n : 2 * n],
                            op=mybir.AluOpType.min,
                        )
                        n //= 2
                    m1 = small.tile([128, 1], f16, tag="m1")
                    nc.vector.tensor_reduce(
                        out=m1,
                        in_=tr[:, :128],
                        axis=mybir.AxisListType.X,
                        op=mybir.AluOpType.min,
                    )
                    mn8 = small.tile([128, 8], f16, tag="mn8")
                    nc.vector.tensor_copy(
                        out=mn8, in_=m1[:, :].broadcast_to([128, 8])
                    )
                    nc.vector.max_index(nearidx[:, t, :], mn8, sh)

                # scatter indices to DRAM in natural g order: g = t*128 + p
                nc.sync.dma_start(
                    out=nidx_d[b].rearrange("(t p) -> p t", p=128),
                    in_=nearidx[:, :, 0],
                )
                # read back wrapped for ap_gather (channels=16):
                # idxw[p, s] = nearidx_dram[s*16 + p]
                idxw = small.tile([16, NP // 16], u16, tag="idxw")
                nc.sync.dma_start(
                    out=idxw, in_=nidx_d[b].rearrange("(s p) -> p s", p=16)
                )
                nn_rep = work.tile([16, NP, 4], f16, tag="nnrep")
                nc.gpsimd.ap_gather(
                    out_ap=nn_rep,
                    in_ap=vn_t,
                    idxs_ap=idxw[:, :].bitcast(mybir.dt.int16),
                    channels=16,
                    num_elems=NG,
                    d=4,
                    num_idxs=NP,
                )
                # roundtrip to get [128, PT, 4] layout (partition p holds g=t*128+p)
                nc.sync.dma_start(out=nn_d[b].unsqueeze(0), in_=nn_rep[0:1])
                nn_t = work.tile([128, PT, 4], f16, tag="nnt")
                nc.sync.dma_start(
                    out=nn_t, in_=nn_d[b].rearrange("(t p) d -> p t d", p=128)
                )

                # renormalize gathered normals
                sq = small.tile([128, PT, 4], f32, tag="sq")
                nc.vector.tensor_mul(sq, nn_t, nn_t)
                ss = small.tile([128, PT], f32, tag="ss")
                nc.vector.tensor_reduce(
                    out=ss, in_=sq, axis=mybir.AxisListType.X, op=mybir.AluOpType.add
                )
                nrm = small.tile([128, PT], f32, tag="nrm")
                nc.scalar.activation(
                    out=nrm, in_=ss, func=mybir.ActivationFunctionType.Sqrt
                )
                nc.vector.tensor_scalar_max(nrm, nrm, 1e-12)
                rs = small.tile([128, PT], f32, tag="rs")
                nc.vector.reciprocal(rs, nrm)
                nnn = work.tile([128, PT, 4], f32, tag="nnn")
                nc.vector.tensor_mul(
                    nnn, nn_t, rs[:, :].unsqueeze(-1).broadcast_to([128, PT, 4])
                )

                # dots with pre-grouped edge vectors, |.|, reduce
                pedge_t = work.tile([128, PT, D, 4], f32, tag="pedge")
                nc.sync.dma_start(out=pedge_t, in_=pedge_d[b])
                prod = work.tile([128, PT, D, 4], f32, tag="prod")
                nc.vector.tensor_mul(
                    prod,
                    pedge_t,
                    nnn[:, :, :].unsqueeze(2).broadcast_to([128, PT, D, 4]),
                )
                dots = small.tile([128, PT, D], f32, tag="dots")
                nc.vector.tensor_reduce(
                    out=dots, in_=prod, axis=mybir.AxisListType.X, op=mybir.AluOpType.add
                )
                absd = small.tile([128, PT, D], f32, tag="absd")
                nc.scalar.activation(
                    out=absd, in_=dots, func=mybir.ActivationFunctionType.Abs
                )
                lp = small.tile([128, 1], f32, tag="lp")
                nc.vector.tensor_reduce(
                    out=lp, in_=absd, axis=mybir.AxisListType.XY, op=mybir.AluOpType.add
                )
                nc.sync.dma_start(out=loss_d[b], in_=lp)

    nc.compile()
    return nc


def _ensure_ntff_hook():
    """Register the axon NTFF profile hook if the image's antenv lacks it."""
    import contextlib
    import ctypes
    import sys
    import types

    try:
        from antenv.axon_hooks import get_axon_ntff_profile_hook  # noqa: F401

        return
    except ImportError:
        pass

    so_path = "/opt/axon/libaxon_pjrt.so"
    if not os.path.exists(so_path):
        return
    lib = ctypes.CDLL(so_path)
    if not hasattr(lib, "axon_start_nrt_profile"):
        return
    lib.axon_start_nrt_profile.argtypes = [
        ctypes.POINTER(ctypes.c_int64),
        ctypes.c_size_t,
    ]
    lib.axon_start_nrt_profile.restype = ctypes.c_int64
    lib.axon_stop_nrt_profile.argtypes = [ctypes.c_char_p]
    lib.axon_stop_nrt_profile.restype = ctypes.c_int64

    @contextlib.contextmanager
    def _hook(output_dir, device_ids):
        import jax

        jax.devices()
        if device_ids:
            ids = (ctypes.c_int64 * len(device_ids))(*device_ids)
            rc = lib.axon_start_nrt_profile(ids, len(device_ids))
        else:
            rc = lib.axon_start_nrt_profile(None, 0)
        if rc != 0:
            raise RuntimeError(f"axon_start_nrt_profile rc={rc}")
        try:
            yield
        finally:
            n = lib.axon_stop_nrt_profile(str(output_dir).encode())
            print(f"ntff profile: {n} file(s) written to {output_dir}", file=sys.stderr)

    mod = types.ModuleType("antenv.axon_hooks")
    holder = {"hook": _hook}
    mod.set_axon_ntff_profile_hook = lambda h: holder.__setitem__("hook", h)
    mod.get_axon_ntff_profile_hook = lambda: holder["hook"]
    sys.modules["antenv.axon_hooks"] = mod
    import antenv

    antenv.axon_hooks = mod


def kernel(pred_vertices, gt_vertices, edges, gt_faces):
    pred = np.asarray(pred_vertices, dtype=np.float32)
    gt = np.asarray(gt_vertices, dtype=np.float32)
    edges = np.asarray(edges)
    faces = np.asarray(gt_faces)

    paug, gaug, vn16, pedge, D, E = _host_prep(pred, gt, edges, faces)

    if D not in _prog_cache:
        _prog_cache[D] = _build_program(D)
    nc = _prog_cache[D]

    in_maps = []
    for c in range(NCORES):
        sl = slice(c * BPC, (c + 1) * BPC)
        in_maps.append(
            {
                "paug": np.ascontiguousarray(paug[sl]),
                "gaug": np.ascontiguousarray(gaug[sl]),
                "vn16": np.ascontiguousarray(vn16[sl]),
                "pedge": np.ascontiguousarray(pedge[sl]),
            }
        )

    from concourse.bass_utils import run_bass_kernel_spmd

    trace = bool(int(os.environ.get("CHAMFER_TRACE", "0")))
    if trace:
        _ensure_ntff_hook()
    res = run_bass_kernel_spmd(
        nc, in_maps, core_ids=list(range(NCORES)), trace=trace
    )
    if trace:
        kernel.last_results = res

    total = np.float64(0.0)
    for r in res.results:
        total += np.float64(r["loss"].sum(dtype=np.float64))
    loss = np.float32(total / (B * E))
    return np.asarray(loss, dtype=np.float32)


# revision 19
# speedup vs baseline: 1.9559x; 1.1992x over previous
"""ChamferNormalLoss Trainium2 kernel.

Strategy (data-parallel over batch, 2 batches per core x 8 cores):
  device per batch:
    - PE matmul computes proxy[p,g] = 2*p.g - |g|^2  (argmax proxy == argmin
      of chamfer d2) into PSUM, 16 p-tiles x 16 g-banks of [128,512].
    - ACT copies each PSUM bank to an fp16 shadow row [128, 8192] in SBUF.
    - DVE InstMax + InstMaxIndex extract the argmax index per pred vertex.
    - near indices roundtrip through DRAM to build the wrapped idx layout,
      gpsimd ap_gather picks the nearest gt vertex normals from an SBUF table.
    - gathered normals are renormalized, dotted against host-pregrouped
      (by edge-endpoint e0) pred edge vectors, |.| and reduced.
  host: topology preprocessing (edge grouping, gt augmentation), gt vertex
  normal table construction, final mean across cores/partitions.
"""

import os

import numpy as np

B, NP, NG = 16, 2048, 8192
NCORES = 8
BPC = B // NCORES  # batches per core
PT = NP // 128  # p tiles per batch (16)
GB = NG // 512  # g banks per batch (16)

_prog_cache = {}


def _host_normals(gt, faces):
    """Per-batch gt vertex normals, matching reference _vertex_normals+_normalize."""
    out = np.zeros((gt.shape[0], NG, 4), dtype=np.float32)
    f0, f1, f2 = faces[:, 0], faces[:, 1], faces[:, 2]
    for b in range(gt.shape[0]):
        gv = gt[b]
        fn = np.cross(gv[f1] - gv[f0], gv[f2] - gv[f0]).astype(np.float32)
        vn = np.zeros((NG, 3), dtype=np.float32)
        np.add.at(vn, f0, fn)
        np.add.at(vn, f1, fn)
        np.add.at(vn, f2, fn)
        n = np.sqrt((vn * vn).sum(-1, keepdims=True))
        vn = vn / np.maximum(n, 1e-12)
        out[b, :, :3] = vn
    return out


def _host_prep(pred, gt, edges, faces):
    e0 = edges[:, 0].astype(np.int64)
    e1 = edges[:, 1].astype(np.int64)
    E = e0.shape[0]

    # group edges by e0 into padded slots
    order = np.argsort(e0, kind="stable")
    e0s, e1s = e0[order], e1[order]
    counts = np.bincount(e0s, minlength=NP)
    D = int(counts.max())
    D = max(D, 1)
    # slot index within each group
    starts = np.zeros(NP, dtype=np.int64)
    starts[1:] = np.cumsum(counts)[:-1]
    slot = np.arange(E) - starts[e0s]

    # pedge[b, p, t, j, d]: edge vectors grouped by e0 vertex
    pedge = np.zeros((B, NP, D, 4), dtype=np.float32)
    ev = pred[:, e0s, :] - pred[:, e1s, :]  # [B, E, 3]
    pedge[:, e0s, slot, :3] = ev
    pedge = pedge.reshape(B, PT, 128, D, 4).transpose(0, 2, 1, 3, 4).copy()

    # bf16x3-split augmented operands so the PE computes
    #   psum = 2 p.g - |g|^2 - |p|^2 = -d2
    # in fast bf16 matmuls with ~f32 accuracy. Each f32 factor is split into
    # three bf16 planes (24 mantissa bits); row r pairs paug[r] * gaug[r].
    import ml_dtypes

    bf16 = ml_dtypes.bfloat16

    def split3(x):
        b0 = x.astype(bf16)
        r1 = x - b0.astype(np.float32)
        b1 = r1.astype(bf16)
        b2 = (r1 - b1.astype(np.float32)).astype(bf16)
        return b0, b1, b2

    KA = 24
    paug = np.zeros((B, KA, NP), dtype=bf16)
    gaug = np.zeros((B, KA, NG), dtype=bf16)
    for k in range(3):
        a0, a1, a2 = split3(2.0 * pred[:, :, k])
        g0, g1, g2 = split3(gt[:, :, k])
        pairs = [(a0, g0), (a0, g1), (a1, g0), (a0, g2), (a2, g0), (a1, g1)]
        for j, (pa, ga) in enumerate(pairs):
            paug[:, 6 * k + j] = pa
            gaug[:, 6 * k + j] = ga
    n0, n1, n2 = split3((gt * gt).sum(-1))
    for j, na in enumerate((n0, n1, n2)):
        paug[:, 18 + j] = np.float32(-1.0).astype(bf16)
        gaug[:, 18 + j] = na
    m0, m1, m2 = split3(-(pred * pred).sum(-1))
    for j, ma in enumerate((m0, m1, m2)):
        paug[:, 21 + j] = ma
        gaug[:, 21 + j] = np.float32(1.0).astype(bf16)

    vn16 = _host_normals(gt, faces).astype(np.float16)  # [B, NG, 4]
    return paug, gaug, vn16, pedge, D, E


def _build_program(D):
    import concourse.bacc as bacc
    import concourse.bass as bass  # noqa: F401
    import concourse.mybir as mybir
    import concourse.tile as tile

    f32 = mybir.dt.float32
    f16 = mybir.dt.float16
    bf16 = mybir.dt.bfloat16
    u16 = mybir.dt.uint16
    KA = 24

    nc = bacc.Bacc("TRN2", target_bir_lowering=False)

    paug_d = nc.dram_tensor("paug", [BPC, KA, NP], bf16, kind="ExternalInput")
    gaug_d = nc.dram_tensor("gaug", [BPC, KA, NG], bf16, kind="ExternalInput")
    vn_d = nc.dram_tensor("vn16", [BPC, NG, 4], f16, kind="ExternalInput")
    pedge_d = nc.dram_tensor("pedge", [BPC, 128, PT, D, 4], f32, kind="ExternalInput")
    loss_d = nc.dram_tensor("loss", [BPC, 128, 1], f32, kind="ExternalOutput")
    nidx_d = nc.dram_tensor("nidx_scratch", [BPC, NP], u16, kind="Internal")
    nn_d = nc.dram_tensor("nn_scratch", [BPC, NP, 4], f16, kind="Internal")

    with tile.TileContext(nc) as tc:
        with (
            tc.tile_pool(name="consts", bufs=1) as consts,
            tc.tile_pool(name="shadow", bufs=2) as shadow_pool,
            tc.tile_pool(name="psum", bufs=2, space="PSUM") as psum_pool,
            tc.tile_pool(name="small", bufs=4) as small,
            tc.tile_pool(name="trees", bufs=1) as trees,
            tc.tile_pool(name="work", bufs=2) as work,
        ):
            for b in range(BPC):
                gaug_t = consts.tile([KA, NG], bf16, tag="gaug")
                nc.sync.dma_start(out=gaug_t, in_=gaug_d[b])
                paug_t = consts.tile([KA, NP], bf16, tag="paug")
                nc.sync.dma_start(out=paug_t, in_=paug_d[b])
                # vn table broadcast to partitions 0..15
                vn_t = consts.tile([16, NG, 4], f16, tag="vn")
                nc.sync.dma_start(
                    out=vn_t, in_=vn_d[b].unsqueeze(0).partition_broadcast(16)
                )

                nearidx = work.tile([128, PT, 8], u16, tag="nearidx")
                for t in range(PT):
                    sh = shadow_pool.tile([128, NG], f16, tag="shadow")
                    lhsT = paug_t[:, t * 128 : (t + 1) * 128]
                    for c4 in range(GB // 4):
                        ps = psum_pool.tile([128, 2048], f32, tag="ps")
                        for q in range(4):
                            c = c4 * 4 + q
                            nc.tensor.matmul(
                                ps[:, q * 512 : (q + 1) * 512],
                                lhsT=lhsT,
                                rhs=gaug_t[:, c * 512 : (c + 1) * 512],
                                start=True,
                                stop=True,
                            )
                        # psum holds -d2; write +d2 to the fp16 shadow
                        nc.scalar.activation(
                            out=sh[:, c4 * 2048 : (c4 + 1) * 2048],
                            in_=ps,
                            func=mybir.ActivationFunctionType.Copy,
                            scale=-1.0,
                        )

                    # argmin via two fp16 min-trees (2 elem/cyc TT folds):
                    #   div tree: min within each 64-wide block  -> trB [128,128]
                    #   mod tree: min over g = j (mod 64)        -> trM [128,64]
                    # then find g* = 64*b* + j* with two tiny FIND_INDEX8 ops.
                    trb = trees.tile([128, NG], f16, tag="treeB")
                    off = 0
                    w = 64
                    src = sh[:, :].rearrange("p (b w) -> p b w", w=64)
                    while w > 1:
                        h = w // 2
                        dst = trb[:, off : off + 128 * h].rearrange(
                            "p (b w) -> p b w", w=h
                        )
                        nc.vector.tensor_tensor(
                            out=dst,
                            in0=src[:, :, :h],
                            in1=src[:, :, h:],
                            op=mybir.AluOpType.min,
                        )
                        src = dst
                        off += 128 * h
                        w = h
                    trB = src.rearrange("p b w -> p (b w)")  # [128, 128]

                    trm = trees.tile([128, NG], f16, tag="treeM")
                    off = 0
                    n = NG // 2
                    msrc = sh[:, :]
                    while n >= 64:
                        dst = trm[:, off : off + n]
                        nc.vector.tensor_tensor(
                            out=dst,
                            in0=msrc[:, :n],
                            in1=msrc[:, n : 2 * n],
                            op=mybir.AluOpType.min,
                        )
                        msrc = dst
                        off += n
                        n //= 2
                    trM = msrc  # [128, 64]

                    m1 = small.tile([128, 1], f16, tag="m1")
                    nc.vector.tensor_reduce(
                        out=m1,
                        in_=trB,
                        axis=mybir.AxisListType.X,
                        op=mybir.AluOpType.min,
                    )
                    mn8 = small.tile([128, 8], f16, tag="mn8")
                    nc.vector.tensor_copy(
                        out=mn8, in_=m1[:, :].broadcast_to([128, 8])
                    )
                    bidx = small.tile([128, 8], u16, tag="bidx")
                    nc.vector.max_index(bidx, mn8, trB)
                    jidx = small.tile([128, 8], u16, tag="jidx")
                    nc.vector.max_index(jidx, mn8, trM)
                    gidx = small.tile([128, 1], u16, tag="gidx")
                    nc.vector.tensor_scalar_mul(gidx, bidx[:, 0:1], 64)
                    nc.vector.tensor_tensor(
                        out=nearidx[:, t, 0:1],
                        in0=gidx,
                        in1=jidx[:, 0:1],
                        op=mybir.AluOpType.add,
                    )

                # scatter indices to DRAM in natural g order: g = t*128 + p
                nc.sync.dma_start(
                    out=nidx_d[b].rearrange("(t p) -> p t", p=128),
                    in_=nearidx[:, :, 0],
                )
                # read back wrapped for ap_gather (channels=16):
                # idxw[p, s] = nearidx_dram[s*16 + p]
                idxw = small.tile([16, NP // 16], u16, tag="idxw")
                nc.sync.dma_start(
                    out=idxw, in_=nidx_d[b].rearrange("(s p) -> p s", p=16)
                )
                nn_rep = work.tile([16, NP, 4], f16, tag="nnrep")
                nc.gpsimd.ap_gather(
                    out_ap=nn_rep,
                    in_ap=vn_t,
                    idxs_ap=idxw[:, :].bitcast(mybir.dt.int16),
                    channels=16,
                    num_elems=NG,
                    d=4,
                    num_idxs=NP,
                )
                # roundtrip to get [128, PT, 4] layout (partition p holds g=t*128+p)
                nc.sync.dma_start(out=nn_d[b].unsqueeze(0), in_=nn_rep[0:1])
                nn_t = work.tile([128, PT, 4], f16, tag="nnt")
                nc.sync.dma_start(
                    out=nn_t, in_=nn_d[b].rearrange("(t p) d -> p t d", p=128)
                )

                # renormalize gathered normals
                sq = small.tile([128, PT, 4], f32, tag="sq")
                nc.vector.tensor_mul(sq, nn_t, nn_t)
                ss = small.tile([128, PT], f32, tag="ss")
                nc.vector.tensor_reduce(
                    out=ss, in_=sq, axis=mybir.AxisListType.X, op=mybir.AluOpType.add
                )
                nrm = small.tile([128, PT], f32, tag="nrm")
                nc.scalar.activation(
                    out=nrm, in_=ss, func=mybir.ActivationFunctionType.Sqrt
                )
                nc.vector.tensor_scalar_max(nrm, nrm, 1e-12)
                rs = small.tile([128, PT], f32, tag="rs")
                nc.vector.reciprocal(rs, nrm)
                nnn = work.tile([128, PT, 4], f32, tag="nnn")
                nc.vector.tensor_mul(
                    nnn, nn_t, rs[:, :].unsqueeze(-1).broadcast_to([128, PT, 4])
                )

                # dots with pre-grouped edge vectors, |.|, reduce
                pedge_t = work.tile([128, PT, D, 4], f32, tag="pedge")
                nc.sync.dma_start(out=pedge_t, in_=pedge_d[b])
                prod = work.tile([128, PT, D, 4], f32, tag="prod")
                nc.vector.tensor_mul(
                    prod,
                    pedge_t,
                    nnn[:, :, :].unsqueeze(2).broadcast_to([128, PT, D, 4]),
                )
                dots = small.tile([128, PT, D], f32, tag="dots")
                nc.vector.tensor_reduce(
                    out=dots, in_=prod, axis=mybir.AxisListType.X, op=mybir.AluOpType.add
                )
                absd = small.tile([128, PT, D], f32, tag="absd")
                nc.scalar.activation(
                    out=absd, in_=dots, func=mybir.ActivationFunctionType.Abs
                )
                lp = small.tile([128, 1], f32, tag="lp")
                nc.vector.tensor_reduce(
                    out=lp, in_=absd, axis=mybir.AxisListType.XY, op=mybir.AluOpType.add
                )
                nc.sync.dma_start(out=loss_d[b], in_=lp)

    nc.compile()
    return nc


def _ensure_ntff_hook():
    """Register the axon NTFF profile hook if the image's antenv lacks it."""
    import contextlib
    import ctypes
    import sys
    import types

    try:
        from antenv.axon_hooks import get_axon_ntff_profile_hook  # noqa: F401

        return
    except ImportError:
        pass

    so_path = "/opt/axon/libaxon_pjrt.so"
    if not os.path.exists(so_path):
        return
    lib = ctypes.CDLL(so_path)
    if not hasattr(lib, "axon_start_nrt_profile"):
        return
    lib.axon_start_nrt_profile.argtypes = [
        ctypes.POINTER(ctypes.c_int64),
        ctypes.c_size_t,
    ]
    lib.axon_start_nrt_profile.restype = ctypes.c_int64
    lib.axon_stop_nrt_profile.argtypes = [ctypes.c_char_p]
    lib.axon_stop_nrt_profile.restype = ctypes.c_int64

    @contextlib.contextmanager
    def _hook(output_dir, device_ids):
        import jax

        jax.devices()
        if device_ids:
            ids = (ctypes.c_int64 * len(device_ids))(*device_ids)
            rc = lib.axon_start_nrt_profile(ids, len(device_ids))
        else:
            rc = lib.axon_start_nrt_profile(None, 0)
        if rc != 0:
            raise RuntimeError(f"axon_start_nrt_profile rc={rc}")
        try:
            yield
        finally:
            n = lib.axon_stop_nrt_profile(str(output_dir).encode())
            print(f"ntff profile: {n} file(s) written to {output_dir}", file=sys.stderr)

    mod = types.ModuleType("antenv.axon_hooks")
    holder = {"hook": _hook}
    mod.set_axon_ntff_profile_hook = lambda h: holder.__setitem__("hook", h)
    mod.get_axon_ntff_profile_hook = lambda: holder["hook"]
    sys.modules["antenv.axon_hooks"] = mod
    import antenv

    antenv.axon_hooks = mod


def kernel(pred_vertices, gt_vertices, edges, gt_faces):
    pred = np.asarray(pred_vertices, dtype=np.float32)
    gt = np.asarray(gt_vertices, dtype=np.float32)
    edges = np.asarray(edges)
    faces = np.asarray(gt_faces)

    paug, gaug, vn16, pedge, D, E = _host_prep(pred, gt, edges, faces)

    if D not in _prog_cache:
        _prog_cache[D] = _build_program(D)
    nc = _prog_cache[D]

    in_maps = []
    for c in range(NCORES):
        sl = slice(c * BPC, (c + 1) * BPC)
        in_maps.append(
            {
                "paug": np.ascontiguousarray(paug[sl]),
                "gaug": np.ascontiguousarray(gaug[sl]),
                "vn16": np.ascontiguousarray(vn16[sl]),
                "pedge": np.ascontiguousarray(pedge[sl]),
            }
        )

    from concourse.bass_utils import run_bass_kernel_spmd

    trace = bool(int(os.environ.get("CHAMFER_TRACE", "0")))
    if trace:
        _ensure_ntff_hook()
    res = run_bass_kernel_spmd(
        nc, in_maps, core_ids=list(range(NCORES)), trace=trace
    )
    if trace:
        kernel.last_results = res

    total = np.float64(0.0)
    for r in res.results:
        total += np.float64(r["loss"].sum(dtype=np.float64))
    loss = np.float32(total / (B * E))
    return np.asarray(loss, dtype=np.float32)


# revision 21
# speedup vs baseline: 1.9847x; 1.0147x over previous
"""ChamferNormalLoss Trainium2 kernel.

Strategy (data-parallel over batch, 2 batches per core x 8 cores):
  device per batch:
    - PE matmul computes proxy[p,g] = 2*p.g - |g|^2  (argmax proxy == argmin
      of chamfer d2) into PSUM, 16 p-tiles x 16 g-banks of [128,512].
    - ACT copies each PSUM bank to an fp16 shadow row [128, 8192] in SBUF.
    - DVE InstMax + InstMaxIndex extract the argmax index per pred vertex.
    - near indices roundtrip through DRAM to build the wrapped idx layout,
      gpsimd ap_gather picks the nearest gt vertex normals from an SBUF table.
    - gathered normals are renormalized, dotted against host-pregrouped
      (by edge-endpoint e0) pred edge vectors, |.| and reduced.
  host: topology preprocessing (edge grouping, gt augmentation), gt vertex
  normal table construction, final mean across cores/partitions.
"""

import os

import numpy as np

B, NP, NG = 16, 2048, 8192
NCORES = 8
BPC = B // NCORES  # batches per core
PT = NP // 128  # p tiles per batch (16)
GB = NG // 512  # g banks per batch (16)

_prog_cache = {}


def _host_normals(gt, faces):
    """Per-batch gt vertex normals, matching reference _vertex_normals+_normalize."""
    out = np.zeros((gt.shape[0], NG, 4), dtype=np.float32)
    f0, f1, f2 = faces[:, 0], faces[:, 1], faces[:, 2]
    for b in range(gt.shape[0]):
        gv = gt[b]
        fn = np.cross(gv[f1] - gv[f0], gv[f2] - gv[f0]).astype(np.float32)
        vn = np.zeros((NG, 3), dtype=np.float32)
        np.add.at(vn, f0, fn)
        np.add.at(vn, f1, fn)
        np.add.at(vn, f2, fn)
        n = np.sqrt((vn * vn).sum(-1, keepdims=True))
        vn = vn / np.maximum(n, 1e-12)
        out[b, :, :3] = vn
    return out


def _host_prep(pred, gt, edges, faces):
    e0 = edges[:, 0].astype(np.int64)
    e1 = edges[:, 1].astype(np.int64)
    E = e0.shape[0]

    # group edges by e0 into padded slots
    order = np.argsort(e0, kind="stable")
    e0s, e1s = e0[order], e1[order]
    counts = np.bincount(e0s, minlength=NP)
    D = int(counts.max())
    D = max(D, 1)
    # slot index within each group
    starts = np.zeros(NP, dtype=np.int64)
    starts[1:] = np.cumsum(counts)[:-1]
    slot = np.arange(E) - starts[e0s]

    # pedge[b, p, t, j, d]: edge vectors grouped by e0 vertex
    pedge = np.zeros((B, NP, D, 4), dtype=np.float32)
    ev = pred[:, e0s, :] - pred[:, e1s, :]  # [B, E, 3]
    pedge[:, e0s, slot, :3] = ev
    pedge = pedge.reshape(B, PT, 128, D, 4).transpose(0, 2, 1, 3, 4).copy()

    # bf16x3-split augmented operands so the PE computes
    #   psum = 2 p.g - |g|^2 - |p|^2 = -d2
    # in fast bf16 matmuls with ~f32 accuracy. Each f32 factor is split into
    # three bf16 planes (24 mantissa bits); row r pairs paug[r] * gaug[r].
    import ml_dtypes

    bf16 = ml_dtypes.bfloat16

    def split3(x):
        b0 = x.astype(bf16)
        r1 = x - b0.astype(np.float32)
        b1 = r1.astype(bf16)
        b2 = (r1 - b1.astype(np.float32)).astype(bf16)
        return b0, b1, b2

    KA = 24
    paug = np.zeros((B, KA, NP), dtype=bf16)
    gaug = np.zeros((B, KA, NG), dtype=bf16)
    for k in range(3):
        a0, a1, a2 = split3(2.0 * pred[:, :, k])
        g0, g1, g2 = split3(gt[:, :, k])
        pairs = [(a0, g0), (a0, g1), (a1, g0), (a0, g2), (a2, g0), (a1, g1)]
        for j, (pa, ga) in enumerate(pairs):
            paug[:, 6 * k + j] = pa
            gaug[:, 6 * k + j] = ga
    n0, n1, n2 = split3((gt * gt).sum(-1))
    for j, na in enumerate((n0, n1, n2)):
        paug[:, 18 + j] = np.float32(-1.0).astype(bf16)
        gaug[:, 18 + j] = na
    m0, m1, m2 = split3(-(pred * pred).sum(-1))
    for j, ma in enumerate((m0, m1, m2)):
        paug[:, 21 + j] = ma
        gaug[:, 21 + j] = np.float32(1.0).astype(bf16)

    vn16 = _host_normals(gt, faces).astype(bf16)  # [B, NG, 4]
    return paug, gaug, vn16, pedge, D, E


def _build_program(D):
    import concourse.bacc as bacc
    import concourse.bass as bass  # noqa: F401
    import concourse.mybir as mybir
    import concourse.tile as tile

    f32 = mybir.dt.float32
    f16 = mybir.dt.float16
    bf16 = mybir.dt.bfloat16
    u16 = mybir.dt.uint16
    KA = 24

    nc = bacc.Bacc("TRN2", target_bir_lowering=False)

    paug_d = nc.dram_tensor("paug", [BPC, KA, NP], bf16, kind="ExternalInput")
    gaug_d = nc.dram_tensor("gaug", [BPC, KA, NG], bf16, kind="ExternalInput")
    vn_d = nc.dram_tensor("vn16", [BPC, NG, 4], bf16, kind="ExternalInput")
    pedge_d = nc.dram_tensor("pedge", [BPC, 128, PT, D, 4], f32, kind="ExternalInput")
    loss_d = nc.dram_tensor("loss", [BPC, 128, 1], f32, kind="ExternalOutput")
    nidx_d = nc.dram_tensor("nidx_scratch", [BPC, NP], u16, kind="Internal")
    nn_d = nc.dram_tensor("nn_scratch", [BPC, NP, 4], bf16, kind="Internal")

    with tile.TileContext(nc) as tc:
        with (
            tc.tile_pool(name="consts", bufs=1) as consts,
            tc.tile_pool(name="aug", bufs=2) as aug,
            tc.tile_pool(name="shadow", bufs=2) as shadow_pool,
            tc.tile_pool(name="psum", bufs=2, space="PSUM") as psum_pool,
            tc.tile_pool(name="small", bufs=4) as small,
            tc.tile_pool(name="trees", bufs=1) as trees,
            tc.tile_pool(name="work", bufs=2) as work,
        ):
            for b in range(BPC):
                gaug_t = aug.tile([KA, NG], bf16, tag="gaug")
                nc.sync.dma_start(out=gaug_t, in_=gaug_d[b])
                paug_t = aug.tile([KA, NP], bf16, tag="paug")
                nc.sync.dma_start(out=paug_t, in_=paug_d[b])
                # vn table broadcast to partitions 0..15
                vn_t = consts.tile([16, NG, 4], bf16, tag="vn")
                nc.sync.dma_start(
                    out=vn_t, in_=vn_d[b].unsqueeze(0).partition_broadcast(16)
                )

                nearidx = work.tile([128, PT, 8], u16, tag="nearidx")
                for t in range(PT):
                    sh = shadow_pool.tile([128, NG], f16, tag="shadow")
                    lhsT = paug_t[:, t * 128 : (t + 1) * 128]
                    for c4 in range(GB // 4):
                        ps = psum_pool.tile([128, 2048], f32, tag="ps")
                        for q in range(4):
                            c = c4 * 4 + q
                            nc.tensor.matmul(
                                ps[:, q * 512 : (q + 1) * 512],
                                lhsT=lhsT,
                                rhs=gaug_t[:, c * 512 : (c + 1) * 512],
                                start=True,
                                stop=True,
                            )
                        # psum holds -d2; write +d2 to the fp16 shadow
                        nc.scalar.activation(
                            out=sh[:, c4 * 2048 : (c4 + 1) * 2048],
                            in_=ps,
                            func=mybir.ActivationFunctionType.Copy,
                            scale=-1.0,
                        )

                    # argmin via two fp16 min-trees (2 elem/cyc TT folds):
                    #   div tree: min within each 64-wide block  -> trB [128,128]
                    #   mod tree: min over g = j (mod 64)        -> trM [128,64]
                    # then find g* = 64*b* + j* with two tiny FIND_INDEX8 ops.
                    trb = trees.tile([128, NG], f16, tag="treeB")
                    off = 0
                    w = 64
                    src = sh[:, :].rearrange("p (b w) -> p b w", w=64)
                    while w > 1:
                        h = w // 2
                        dst = trb[:, off : off + 128 * h].rearrange(
                            "p (b w) -> p b w", w=h
                        )
                        nc.vector.tensor_tensor(
                            out=dst,
                            in0=src[:, :, :h],
                            in1=src[:, :, h:],
                            op=mybir.AluOpType.min,
                        )
                        src = dst
                        off += 128 * h
                        w = h
                    trB = src.rearrange("p b w -> p (b w)")  # [128, 128]

                    trm = trees.tile([128, NG], f16, tag="treeM")
                    off = 0
                    n = NG // 2
                    msrc = sh[:, :]
                    while n >= 64:
                        dst = trm[:, off : off + n]
                        nc.vector.tensor_tensor(
                            out=dst,
                            in0=msrc[:, :n],
                            in1=msrc[:, n : 2 * n],
                            op=mybir.AluOpType.min,
                        )
                        msrc = dst
                        off += n
                        n //= 2
                    trM = msrc  # [128, 64]

                    m1 = small.tile([128, 1], f16, tag="m1")
                    nc.vector.tensor_reduce(
                        out=m1,
                        in_=trB,
                        axis=mybir.AxisListType.X,
                        op=mybir.AluOpType.min,
                    )
                    m8 = m1[:, :].broadcast_to([128, 8])
                    bidx = small.tile([128, 8], u16, tag="bidx")
                    nc.vector.max_index(bidx, m8, trB)
                    jidx = small.tile([128, 8], u16, tag="jidx")
                    nc.vector.max_index(jidx, m8, trM)
                    nc.vector.scalar_tensor_tensor(
                        out=nearidx[:, t, 0:1],
                        in0=bidx[:, 0:1],
                        scalar=64,
                        in1=jidx[:, 0:1],
                        op0=mybir.AluOpType.mult,
                        op1=mybir.AluOpType.add,
                    )

                # scatter indices to DRAM in natural g order: g = t*128 + p
                nc.sync.dma_start(
                    out=nidx_d[b].rearrange("(t p) -> p t", p=128),
                    in_=nearidx[:, :, 0],
                )
                # read back wrapped for ap_gather (channels=16):
                # idxw[p, s] = nearidx_dram[s*16 + p]
                idxw = small.tile([16, NP // 16], u16, tag="idxw")
                nc.sync.dma_start(
                    out=idxw, in_=nidx_d[b].rearrange("(s p) -> p s", p=16)
                )
                nn_rep = trees.tile([16, NP, 4], bf16, tag="nnrep")
                nc.gpsimd.ap_gather(
                    out_ap=nn_rep,
                    in_ap=vn_t,
                    idxs_ap=idxw[:, :].bitcast(mybir.dt.int16),
                    channels=16,
                    num_elems=NG,
                    d=4,
                    num_idxs=NP,
                )
                # roundtrip to get [128, PT, 4] layout (partition p holds g=t*128+p)
                nc.sync.dma_start(out=nn_d[b].unsqueeze(0), in_=nn_rep[0:1])
                nn_t = work.tile([128, PT, 4], bf16, tag="nnt")
                nc.sync.dma_start(
                    out=nn_t, in_=nn_d[b].rearrange("(t p) d -> p t d", p=128)
                )

                # renormalize gathered normals
                sq = small.tile([128, PT, 4], f32, tag="sq")
                nc.vector.tensor_mul(sq, nn_t, nn_t)
                ss = small.tile([128, PT], f32, tag="ss")
                nc.vector.tensor_reduce(
                    out=ss, in_=sq, axis=mybir.AxisListType.X, op=mybir.AluOpType.add
                )
                nrm = small.tile([128, PT], f32, tag="nrm")
                nc.scalar.activation(
                    out=nrm, in_=ss, func=mybir.ActivationFunctionType.Sqrt
                )
                nc.vector.tensor_scalar_max(nrm, nrm, 1e-12)
                rs = small.tile([128, PT], f32, tag="rs")
                nc.vector.reciprocal(rs, nrm)
                nnn = work.tile([128, PT, 4], f32, tag="nnn")
                nc.vector.tensor_mul(
                    nnn, nn_t, rs[:, :].unsqueeze(-1).broadcast_to([128, PT, 4])
                )

                # dots with pre-grouped edge vectors, |.|, reduce
                pedge_t = work.tile([128, PT, D, 4], f32, tag="pedge")
                nc.sync.dma_start(out=pedge_t, in_=pedge_d[b])
                prod = work.tile([128, PT, D, 4], f32, tag="prod")
                nc.vector.tensor_mul(
                    prod,
                    pedge_t,
                    nnn[:, :, :].unsqueeze(2).broadcast_to([128, PT, D, 4]),
                )
                dots = small.tile([128, PT, D], f32, tag="dots")
                nc.vector.tensor_reduce(
                    out=dots, in_=prod, axis=mybir.AxisListType.X, op=mybir.AluOpType.add
                )
                absd = small.tile([128, PT, D], f32, tag="absd")
                nc.scalar.activation(
                    out=absd, in_=dots, func=mybir.ActivationFunctionType.Abs
                )
                lp = small.tile([128, 1], f32, tag="lp")
                nc.vector.tensor_reduce(
                    out=lp, in_=absd, axis=mybir.AxisListType.XY, op=mybir.AluOpType.add
                )
                nc.sync.dma_start(out=loss_d[b], in_=lp)

    nc.compile()
    return nc


def _ensure_ntff_hook():
    """Register the axon NTFF profile hook if the image's antenv lacks it."""
    import contextlib
    import ctypes
    import sys
    import types

    try:
        from antenv.axon_hooks import get_axon_ntff_profile_hook  # noqa: F401

        return
    except ImportError:
        pass

    so_path = "/opt/axon/libaxon_pjrt.so"
    if not os.path.exists(so_path):
        return
    lib = ctypes.CDLL(so_path)
    if not hasattr(lib, "axon_start_nrt_profile"):
        return
    lib.axon_start_nrt_profile.argtypes = [
        ctypes.POINTER(ctypes.c_int64),
        ctypes.c_size_t,
    ]
    lib.axon_start_nrt_profile.restype = ctypes.c_int64
    lib.axon_stop_nrt_profile.argtypes = [ctypes.c_char_p]
    lib.axon_stop_nrt_profile.restype = ctypes.c_int64

    @contextlib.contextmanager
    def _hook(output_dir, device_ids):
        import jax

        jax.devices()
        if device_ids:
            ids = (ctypes.c_int64 * len(device_ids))(*device_ids)
            rc = lib.axon_start_nrt_profile(ids, len(device_ids))
        else:
            rc = lib.axon_start_nrt_profile(None, 0)
        if rc != 0:
            raise RuntimeError(f"axon_start_nrt_profile rc={rc}")
        try:
            yield
        finally:
            n = lib.axon_stop_nrt_profile(str(output_dir).encode())
            print(f"ntff profile: {n} file(s) written to {output_dir}", file=sys.stderr)

    mod = types.ModuleType("antenv.axon_hooks")
    holder = {"hook": _hook}
    mod.set_axon_ntff_profile_hook = lambda h: holder.__setitem__("hook", h)
    mod.get_axon_ntff_profile_hook = lambda: holder["hook"]
    sys.modules["antenv.axon_hooks"] = mod
    import antenv

    antenv.axon_hooks = mod


def kernel(pred_vertices, gt_vertices, edges, gt_faces):
    pred = np.asarray(pred_vertices, dtype=np.float32)
    gt = np.asarray(gt_vertices, dtype=np.float32)
    edges = np.asarray(edges)
    faces = np.asarray(gt_faces)

    paug, gaug, vn16, pedge, D, E = _host_prep(pred, gt, edges, faces)

    if D not in _prog_cache:
        _prog_cache[D] = _build_program(D)
    nc = _prog_cache[D]

    in_maps = []
    for c in range(NCORES):
        sl = slice(c * BPC, (c + 1) * BPC)
        in_maps.append(
            {
                "paug": np.ascontiguousarray(paug[sl]),
                "gaug": np.ascontiguousarray(gaug[sl]),
                "vn16": np.ascontiguousarray(vn16[sl]),
                "pedge": np.ascontiguousarray(pedge[sl]),
            }
        )

    from concourse.bass_utils import run_bass_kernel_spmd

    trace = bool(int(os.environ.get("CHAMFER_TRACE", "0")))
    if trace:
        _ensure_ntff_hook()
    res = run_bass_kernel_spmd(
        nc, in_maps, core_ids=list(range(NCORES)), trace=trace
    )
    if trace:
        kernel.last_results = res

    total = np.float64(0.0)
    for r in res.results:
        total += np.float64(r["loss"].sum(dtype=np.float64))
    loss = np.float32(total / (B * E))
    return np.asarray(loss, dtype=np.float32)
